# revision 7
# baseline (speedup 1.0000x reference)
"""Trainium2 Bass kernel for nn_CPLinear (CP-decomposed QKV projection with RoPE).

Computes, for x:(2,4096,2048) and CP-factor weights:
    A_t = x @ W_A_t  (per-token head coefficients),  B_t = x @ W_B_t (shared bases)
    q = einsum('bshr,bsrd->bshd', A_q, rope(B_q)) / 12
    k = A_k * rope(B_k)   (rank-1)
    v = A_v * B_v         (rank-1)

Strategy (8 cores, data-parallel over the 8192 tokens, 1024 tokens/core):
  - All 6 projections fused into one [2048 x 2016] bf16 matmul (PE), with the
    1/12 scale and (h,r)->(r,h) reorder folded into W_A_q host-side.
  - x is uploaded as bf16 and loaded transposed via the DMA xbar transpose so
    the contraction dim lands on partitions with no on-chip transposes.
  - RoPE applied to B_q/B_k with bf16 tensor_tensor ops (cos/sin tables are
    host-precomputed per-token inputs, replicated x12 along r).
  - The per-token rank-12 contraction for q runs on the PE as a block-diagonal
    matmul: 8 tokens/matmul, K=96=(8 tokens x 12 r), M=128=(8 tokens x 16 h),
    N=128=d. Operands are built by partition-interleaving scatter DMAs.
  - k/v are per-partition-scalar broadcasts (DVE tensor_scalar / ACT activation).
  - Outputs are written bf16 and widened to fp32 on the host.
"""

import sys

for _p in ("/opt/trn_rl_repo",):
    if _p not in sys.path:
        sys.path.insert(0, _p)

import numpy as np
import ml_dtypes

BF16 = ml_dtypes.bfloat16

SH = 1024          # tokens per core
H = 2048           # hidden
KT = H // 128      # 16 k-tiles
NT = SH // 128     # 8 token tiles per core
NOUT = 2016        # fused projection output width
NH, HD, RQ = 16, 128, 12

_CACHE = {}


def make_nc():
    import concourse.bacc as bacc
    from concourse import mybir

    dt = mybir.dt

    nc = bacc.Bacc(
        "TRN2",
        target_bir_lowering=False,
        debug=False,
        enable_asserts=False,
        num_devices=8,
    )

    x_d = nc.dram_tensor("x", (SH, H), dt.bfloat16, kind="ExternalInput")
    w_d = nc.dram_tensor("w", (KT, 128, NOUT), dt.bfloat16, kind="ExternalInput")
    cos_d = nc.dram_tensor("cosr", (SH, 768), dt.bfloat16, kind="ExternalInput")
    sin_d = nc.dram_tensor("sinr", (SH, 768), dt.bfloat16, kind="ExternalInput")
    q_d = nc.dram_tensor("q", (SH, NH, HD), dt.bfloat16, kind="ExternalOutput")
    k_d = nc.dram_tensor("k", (SH, NH * HD), dt.bfloat16, kind="ExternalOutput")
    v_d = nc.dram_tensor("v", (SH, NH * HD), dt.bfloat16, kind="ExternalOutput")
    return nc, (x_d, w_d, cos_d, sin_d, q_d, k_d, v_d)


def build_body(nc, tc, tensors):
    from contextlib import ExitStack

    from concourse import mybir

    dt = mybir.dt
    x_d, w_d, cos_d, sin_d, q_d, k_d, v_d = tensors

    with ExitStack() as ctx:
        P = ctx.enter_context
        const_pool = P(tc.tile_pool(name="const", bufs=1))
        w_sb = const_pool.tile([128, KT * NOUT], dt.bfloat16, tag="w_sb")
        cos_sb = const_pool.tile([128, NT * 768], dt.bfloat16, tag="cos_sb")
        sin_sb = const_pool.tile([128, NT * 768], dt.bfloat16, tag="sin_sb")
        xT = const_pool.tile([128, KT * SH], dt.bfloat16, tag="xT")
        # ping-pong block-diagonal lhsT holders for the q contraction
        lhs0 = const_pool.tile([128, 2048], dt.bfloat16, tag="lhs0")
        lhs1 = const_pool.tile([128, 2048], dt.bfloat16, tag="lhs1")

        # constant loads (ACT HWDGE ring) — keep SBUF APs partition-leading
        for kk in range(KT):
            nc.scalar.dma_start(
                out=w_sb[:, kk * NOUT : (kk + 1) * NOUT], in_=w_d[kk]
            )
        for t in range(NT):
            nc.scalar.dma_start(
                out=cos_sb[:, t * 768 : (t + 1) * 768],
                in_=cos_d[t * 128 : (t + 1) * 128, :],
            )
            nc.scalar.dma_start(
                out=sin_sb[:, t * 768 : (t + 1) * 768],
                in_=sin_d[t * 128 : (t + 1) * 128, :],
            )
        nc.vector.memset(lhs0[:], 0.0)
        nc.vector.memset(lhs1[:], 0.0)
        # transposed x load (SP HWDGE ring, xbar transpose)
        for kk in range(KT):
            nc.sync.dma_start(
                out=xT[:, kk * SH : (kk + 1) * SH],
                in_=x_d[:, kk * 128 : (kk + 1) * 128],
                transpose=True,
            )

        ps_pool = P(tc.tile_pool(name="ps1", bufs=1, space="PSUM"))
        psq_pool = P(tc.tile_pool(name="psq", bufs=2, space="PSUM"))
        bq_pool = P(tc.tile_pool(name="bq", bufs=2))
        bqr_pool = P(tc.tile_pool(name="bqr", bufs=2))
        tmp_pool = P(tc.tile_pool(name="tmp", bufs=2))
        bdr_pool = P(tc.tile_pool(name="bdr", bufs=2))
        small_pool = P(tc.tile_pool(name="small", bufs=2))
        out_pool = P(tc.tile_pool(name="outs", bufs=2))
        dram_pool = P(tc.tile_pool(name="scr", bufs=2, space="DRAM"))

        for it in range(NT):
            t0 = it * 128
            lhs = lhs0 if it % 2 == 0 else lhs1

            # ---- fused projection: ps = x_tile @ W_all (fp32 PSUM) ----
            ps = ps_pool.tile([128, 2048], dt.float32, tag="ps")
            for kk in range(KT):
                lh = xT[:, kk * SH + t0 : kk * SH + t0 + 128]
                wb = kk * NOUT
                st = kk == 0
                sp = kk == KT - 1
                nc.tensor.matmul(
                    ps[:, 0:480], lh, w_sb[:, wb : wb + 480], start=st, stop=sp
                )
                nc.tensor.matmul(
                    ps[:, 512:1024], lh, w_sb[:, wb + 480 : wb + 992],
                    start=st, stop=sp,
                )
                nc.tensor.matmul(
                    ps[:, 1024:1536], lh, w_sb[:, wb + 992 : wb + 1504],
                    start=st, stop=sp,
                )
                nc.tensor.matmul(
                    ps[:, 1536:2048], lh, w_sb[:, wb + 1504 : wb + 2016],
                    start=st, stop=sp,
                )

            # ---- PSUM evictions (ACT) ----
            a_sb = small_pool.tile([128, 192], dt.bfloat16, tag="a_sb")
            ak_sb = small_pool.tile([128, 16], dt.float32, tag="ak_sb")
            av_sb = small_pool.tile([128, 16], dt.float32, tag="av_sb")
            bk_sb = small_pool.tile([128, 128], dt.bfloat16, tag="bk_sb")
            bkr_sb = small_pool.tile([128, 128], dt.bfloat16, tag="bkr_sb")
            bv_sb = small_pool.tile([128, 128], dt.bfloat16, tag="bv_sb")
            bq_sb = bq_pool.tile([128, 1536], dt.bfloat16, tag="bq_sb")
            nc.scalar.copy(a_sb[:], ps[:, 0:192])
            nc.scalar.copy(ak_sb[:], ps[:, 192:208])
            nc.scalar.copy(av_sb[:], ps[:, 208:224])
            nc.scalar.copy(bk_sb[:], ps[:, 224:352])
            nc.scalar.copy(bv_sb[:], ps[:, 352:480])
            nc.scalar.copy(bq_sb[:], ps[:, 512:2048])

            # ---- RoPE on B_q (DVE, bf16) ----
            bqr = bqr_pool.tile([128, 1536], dt.bfloat16, tag="bqr_t")
            t_a = tmp_pool.tile([128, 768], dt.bfloat16, tag="t_a")
            t_b = tmp_pool.tile([128, 768], dt.bfloat16, tag="t_b")
            bqv = bq_sb[:].rearrange("p (r two d) -> p r two d", r=RQ, two=2)
            bqrv = bqr[:].rearrange("p (r two d) -> p r two d", r=RQ, two=2)
            cos_t = cos_sb[:, it * 768 : (it + 1) * 768].rearrange(
                "p (r d) -> p r d", r=RQ
            )
            sin_t = sin_sb[:, it * 768 : (it + 1) * 768].rearrange(
                "p (r d) -> p r d", r=RQ
            )
            tav = t_a[:].rearrange("p (r d) -> p r d", r=RQ)
            tbv = t_b[:].rearrange("p (r d) -> p r d", r=RQ)
            p_lo = bqv[:, :, 0]
            p_hi = bqv[:, :, 1]
            nc.vector.tensor_mul(tav, p_lo, cos_t)
            nc.vector.tensor_mul(tbv, p_hi, sin_t)
            nc.vector.tensor_sub(bqrv[:, :, 0], tav, tbv)
            nc.vector.tensor_mul(tav, p_hi, cos_t)
            nc.vector.tensor_mul(tbv, p_lo, sin_t)
            nc.vector.tensor_add(bqrv[:, :, 1], tav, tbv)

            # ---- RoPE on B_k (DVE, bf16) ----
            tk_a = small_pool.tile([128, 64], dt.bfloat16, tag="tk_a")
            tk_b = small_pool.tile([128, 64], dt.bfloat16, tag="tk_b")
            bkv = bk_sb[:].rearrange("p (two d) -> p two d", two=2)
            bkrv = bkr_sb[:].rearrange("p (two d) -> p two d", two=2)
            cos_k = cos_sb[:, it * 768 : it * 768 + 64]
            sin_k = sin_sb[:, it * 768 : it * 768 + 64]
            nc.vector.tensor_mul(tk_a[:], bkv[:, 0], cos_k)
            nc.vector.tensor_mul(tk_b[:], bkv[:, 1], sin_k)
            nc.vector.tensor_sub(bkrv[:, 0], tk_a[:], tk_b[:])
            nc.vector.tensor_mul(tk_a[:], bkv[:, 1], cos_k)
            nc.vector.tensor_mul(tk_b[:], bkv[:, 0], sin_k)
            nc.vector.tensor_add(bkrv[:, 1], tk_a[:], tk_b[:])

            # ---- scatter A', roped B_q into block-diagonal layout ----
            # Bounce through DRAM: SBUF-side APs of a partition-interleave
            # must stay partition-leading (dep tracker limitation), so the
            # reorder happens on the DRAM side of a read-back.
            scr_a = dram_pool.tile([128, 192], dt.bfloat16, tag="scr_a")
            scr_b = dram_pool.tile([128, 1536], dt.bfloat16, tag="scr_b")
            nc.scalar.dma_start(out=scr_a[:], in_=a_sb[:])
            nc.scalar.dma_start(out=scr_b[:], in_=bqr[:])
            bdr = bdr_pool.tile([128, 2048], dt.bfloat16, tag="bdr_t")
            sa_v = scr_a[:].rearrange("(g t) (r h) -> t r g h", t=8, r=RQ)
            sb_v = scr_b[:].rearrange("(g t) (r d) -> t r g d", t=8, r=RQ)
            l_v = lhs[0:96, :].rearrange("(t r) (g c) -> t r g c", t=8, g=16)
            d_v = bdr[0:96, :].rearrange("(t r) (g d) -> t r g d", t=8, g=16)
            for t in range(8):
                nc.scalar.dma_start(
                    out=l_v[t][:, :, t * 16 : (t + 1) * 16], in_=sa_v[t]
                )
                nc.scalar.dma_start(out=d_v[t], in_=sb_v[t])

            # ---- q: block-diagonal matmuls (PE) + PSUM evict (DVE) ----
            qsb = out_pool.tile([128, 2048], dt.bfloat16, tag="qsb")
            for gq in range(4):
                qp = psq_pool.tile([128, 512], dt.float32, tag="qp")
                for j in range(4):
                    g = gq * 4 + j
                    nc.tensor.matmul(
                        qp[:, j * 128 : (j + 1) * 128],
                        lhs[0:96, g * 128 : (g + 1) * 128],
                        bdr[0:96, g * 128 : (g + 1) * 128],
                        start=True,
                        stop=True,
                    )
                nc.vector.tensor_copy(qsb[:, gq * 512 : (gq + 1) * 512], qp[:])

            # ---- k (DVE tensor_scalar), v (ACT activation) ----
            ksb = out_pool.tile([128, 2048], dt.bfloat16, tag="ksb")
            vsb = out_pool.tile([128, 2048], dt.bfloat16, tag="vsb")
            for h in range(NH):
                nc.vector.tensor_scalar_mul(
                    ksb[:, h * 128 : (h + 1) * 128], bkr_sb[:], ak_sb[:, h : h + 1]
                )
                nc.scalar.mul(
                    vsb[:, h * 128 : (h + 1) * 128], bv_sb[:], av_sb[:, h : h + 1]
                )

            # ---- outputs (SP HWDGE ring) ----
            nc.sync.dma_start(
                out=q_d[t0 : t0 + 128].rearrange("(g t) h d -> (t h) g d", g=16),
                in_=qsb[:].rearrange("p (g d) -> p g d", g=16),
            )
            nc.sync.dma_start(out=k_d[t0 : t0 + 128, :], in_=ksb[:])
            nc.sync.dma_start(out=v_d[t0 : t0 + 128, :], in_=vsb[:])


def build_program():
    import concourse.tile as tile

    nc, tensors = make_nc()
    with tile.TileContext(nc) as tc:
        build_body(nc, tc, tensors)
    nc.compile()
    return nc


def _get_program():
    if "nc" not in _CACHE:
        _CACHE["nc"] = build_program()
    return _CACHE["nc"]


def make_in_maps(x, W_A_q, W_B_q, W_A_k, W_B_k, W_A_v, W_B_v):
    """Shard + preprocess full inputs into per-core input maps."""
    x = np.asarray(x)
    B, S, Hh = x.shape
    x2 = np.ascontiguousarray(x.reshape(B * S, Hh))

    # fold the 1/RQ scale and the (h,r)->(r,h) column reorder into W_A_q
    WAq = np.asarray(W_A_q).reshape(Hh, NH, RQ).transpose(0, 2, 1).reshape(
        Hh, NH * RQ
    ) / np.float32(RQ)
    Wall = np.concatenate(
        [
            WAq,
            np.asarray(W_A_k),
            np.asarray(W_A_v),
            np.asarray(W_B_k),
            np.asarray(W_B_v),
            np.asarray(W_B_q),
        ],
        axis=1,
    )
    assert Wall.shape == (Hh, NOUT)
    Wt = np.ascontiguousarray(Wall.reshape(KT, 128, NOUT)).astype(BF16)

    inv = 1.0 / (10000.0 ** (np.arange(0, HD, 2, dtype=np.float32) / HD))
    ang = np.arange(S, dtype=np.float32)[:, None] * inv[None, :]
    cos_rep = np.ascontiguousarray(np.tile(np.cos(ang), (1, RQ))).astype(BF16)
    sin_rep = np.ascontiguousarray(np.tile(np.sin(ang), (1, RQ))).astype(BF16)

    in_maps = []
    for i in range(8):
        tok0 = i * SH
        pos = np.arange(tok0, tok0 + SH) % S
        in_maps.append(
            {
                "x": np.ascontiguousarray(x2[tok0 : tok0 + SH]).astype(BF16),
                "w": Wt,
                "cosr": np.ascontiguousarray(cos_rep[pos]),
                "sinr": np.ascontiguousarray(sin_rep[pos]),
            }
        )
    return in_maps, (B, S)


def assemble_outputs(results, B, S):
    q = np.concatenate(
        [results[i]["q"].astype(np.float32) for i in range(8)], axis=0
    ).reshape(B, S, NH, HD)
    k = np.concatenate(
        [results[i]["k"].astype(np.float32) for i in range(8)], axis=0
    ).reshape(B, S, NH, HD)
    v = np.concatenate(
        [results[i]["v"].astype(np.float32) for i in range(8)], axis=0
    ).reshape(B, S, NH, HD)
    return q, k, v


def kernel(x, W_A_q, W_B_q, W_A_k, W_B_k, W_A_v, W_B_v):
    from concourse.bass_utils import run_bass_kernel_spmd

    nc = _get_program()
    in_maps, (B, S) = make_in_maps(x, W_A_q, W_B_q, W_A_k, W_B_k, W_A_v, W_B_v)
    res = run_bass_kernel_spmd(nc, in_maps, list(range(8))).results
    return assemble_outputs(res, B, S)


# revision 22
# speedup vs baseline: 2.8312x; 2.8312x over previous
"""Trainium2 Bass kernel for nn_CPLinear (CP-decomposed QKV projection with RoPE).

Computes, for x:(2,4096,2048) and CP-factor weights:
    A_t = x @ W_A_t  (per-token head coefficients),  B_t = x @ W_B_t (shared bases)
    q = einsum('bshr,bsrd->bshd', A_q, rope(B_q)) / 12
    k = A_k * rope(B_k)   (rank-1)
    v = A_v * B_v         (rank-1)

Strategy (8 cores, data-parallel over the 8192 tokens, 1024 tokens/core):
  - All 6 projections fused into one [2048 x 2016] bf16 matmul (PE), with the
    1/12 scale and (h,r)->(r,h) reorder folded into W_A_q host-side.
  - x is uploaded as bf16 and loaded transposed via the DMA xbar transpose so
    the contraction dim lands on partitions with no on-chip transposes.
  - RoPE applied to B_q/B_k with bf16 tensor_tensor ops (cos/sin tables are
    host-precomputed per-token inputs, replicated x12 along r).
  - The per-token rank-12 contraction for q runs on the PE as a block-diagonal
    matmul: 8 tokens/matmul, K=96=(8 tokens x 12 r), M=128=(8 tokens x 16 h),
    N=128=d. Operands are built by partition-interleaving scatter DMAs.
  - k/v are per-partition-scalar broadcasts (DVE tensor_scalar / ACT activation).
  - Outputs are written bf16 and widened to fp32 on the host.
"""

import sys

for _p in ("/opt/trn_rl_repo",):
    if _p not in sys.path:
        sys.path.insert(0, _p)

import numpy as np
import ml_dtypes

BF16 = ml_dtypes.bfloat16

SH = 1024          # tokens per core
H = 2048           # hidden
KT = H // 128      # 16 k-tiles
NT = SH // 128     # 8 token tiles per core
NOUT = 2016        # fused projection output width
NH, HD, RQ = 16, 128, 12

_CACHE = {}


def make_nc():
    import concourse.bacc as bacc
    from concourse import mybir

    dt = mybir.dt

    nc = bacc.Bacc(
        "TRN2",
        target_bir_lowering=False,
        debug=False,
        enable_asserts=False,
        num_devices=8,
    )

    x_d = nc.dram_tensor("x", (H, SH), dt.bfloat16, kind="ExternalInput")  # pre-transposed host-side
    w_d = nc.dram_tensor("w", (KT, 128, NOUT), dt.bfloat16, kind="ExternalInput")
    cos_d = nc.dram_tensor("cosr", (SH, 768), dt.bfloat16, kind="ExternalInput")
    sin_d = nc.dram_tensor("sinr", (SH, 768), dt.bfloat16, kind="ExternalInput")
    q_d = nc.dram_tensor("q", (SH, NH, HD), dt.bfloat16, kind="ExternalOutput")
    k_d = nc.dram_tensor("k", (SH, NH * HD), dt.bfloat16, kind="ExternalOutput")
    v_d = nc.dram_tensor("v", (SH, NH * HD), dt.bfloat16, kind="ExternalOutput")
    return nc, (x_d, w_d, cos_d, sin_d, q_d, k_d, v_d)


def build_body(nc, tc, tensors):
    from contextlib import ExitStack

    from concourse import mybir

    dt = mybir.dt
    x_d, w_d, cos_d, sin_d, q_d, k_d, v_d = tensors

    with ExitStack() as ctx:
        P = ctx.enter_context
        const_pool = P(tc.tile_pool(name="const", bufs=1))
        w_sb = const_pool.tile([128, KT * NOUT], dt.bfloat16, tag="w_sb")
        cos_sb = const_pool.tile([128, NT * 768], dt.bfloat16, tag="cos_sb")
        sin_sb = const_pool.tile([128, NT * 768], dt.bfloat16, tag="sin_sb")
        xT = const_pool.tile([128, KT * SH], dt.bfloat16, tag="xT")
        # ping-pong block-diagonal lhsT holders for the q contraction
        lhs0 = const_pool.tile([128, 2048], dt.bfloat16, tag="lhs0")
        lhs1 = const_pool.tile([128, 2048], dt.bfloat16, tag="lhs1")

        # constant loads — quarter-merged DMAs (pipelines the first matmuls
        # while keeping the HWDGE instruction count low); SBUF dims stay
        # partition-first, the reorder lives on the DRAM side of the AP.
        w_v = w_sb[:].rearrange("p (k n) -> p k n", k=KT)
        wd_v = w_d[:].rearrange("k p n -> p k n")
        x_v = xT[:].rearrange("p (k t) -> p k t", k=KT)
        xd_v = x_d[:].rearrange("(k p) t -> p k t", p=128)
        for qtr in range(4):
            sl = slice(qtr * 4, (qtr + 1) * 4)
            nc.scalar.dma_start(out=w_v[:, sl], in_=wd_v[:, sl])
            nc.sync.dma_start(out=x_v[:, sl], in_=xd_v[:, sl])
            if qtr == 0:
                nc.scalar.dma_start(
                    out=cos_sb[:].rearrange("p (t n) -> p t n", t=NT),
                    in_=cos_d[:].rearrange("(t p) n -> p t n", p=128),
                )
                nc.scalar.dma_start(
                    out=sin_sb[:].rearrange("p (t n) -> p t n", t=NT),
                    in_=sin_d[:].rearrange("(t p) n -> p t n", p=128),
                )
        # bdr ping-pong buffers: persistent + memset once (shields the sim's
        # conservative write-coverage tracking for the merged readback AP)
        bdr0 = const_pool.tile([128, 2048], dt.bfloat16, tag="bdr0")
        bdr1 = const_pool.tile([128, 2048], dt.bfloat16, tag="bdr1")
        nc.vector.memset(lhs0[:], 0.0)
        nc.vector.memset(lhs1[:], 0.0)
        nc.vector.memset(bdr0[:], 0.0)
        nc.vector.memset(bdr1[:], 0.0)

        psa_pool = P(tc.tile_pool(name="psa", bufs=2, space="PSUM"))
        psb_pool = P(tc.tile_pool(name="psb", bufs=1, space="PSUM"))
        psq_pool = P(tc.tile_pool(name="psq", bufs=2, space="PSUM"))
        bq_pool = P(tc.tile_pool(name="bq", bufs=2))
        bqr_pool = P(tc.tile_pool(name="bqr", bufs=2))
        tmp_pool = P(tc.tile_pool(name="tmp", bufs=2))
        small_pool = P(tc.tile_pool(name="small", bufs=3))
        out_pool = P(tc.tile_pool(name="outs", bufs=2))
        dram_pool = P(tc.tile_pool(name="scr", bufs=2, space="DRAM"))

        from concourse.ap import AP

        # per-tile state carried across the software-pipeline stages
        state = {}

        def produce(it):
            """step-1 projection, evictions, RoPE, scatter for tile `it`."""
            t0 = it * 128
            lhs = lhs0 if it % 2 == 0 else lhs1
            bdr = bdr0 if it % 2 == 0 else bdr1

            ps_a = psa_pool.tile([128, 512], dt.float32, tag="ps_a")
            ps_b = psb_pool.tile([128, 1536], dt.float32, tag="ps_b")
            for kk in range(KT):
                lh = xT[:, kk * SH + t0 : kk * SH + t0 + 128]
                wb = kk * NOUT
                st = kk == 0
                sp = kk == KT - 1
                nc.tensor.matmul(
                    ps_a[:, 0:480], lh, w_sb[:, wb : wb + 480], start=st, stop=sp
                )
                for c in range(3):
                    nc.tensor.matmul(
                        ps_b[:, c * 512 : (c + 1) * 512],
                        lh,
                        w_sb[:, wb + 480 + c * 512 : wb + 480 + (c + 1) * 512],
                        start=st,
                        stop=sp,
                    )

            # ---- PSUM evictions (ACT) ----
            ak_sb = small_pool.tile([128, 16], dt.float32, tag="ak_sb")
            av_sb = small_pool.tile([128, 16], dt.float32, tag="av_sb")
            bk_sb = small_pool.tile([128, 128], dt.bfloat16, tag="bk_sb")
            bkr_sb = small_pool.tile([128, 128], dt.bfloat16, tag="bkr_sb")
            bv_sb = small_pool.tile([128, 128], dt.bfloat16, tag="bv_sb")
            bq_sb = bq_pool.tile([128, 1536], dt.bfloat16, tag="bq_sb")
            # bqr holds roped B_q (cols 0:1536) and A' (cols 1536:1728) so the
            # DRAM bounce is a single DMA
            bqr = bqr_pool.tile([128, 1728], dt.bfloat16, tag="bqr_t")
            nc.scalar.copy(bqr[:, 1536:1728], ps_a[:, 0:192])
            nc.scalar.copy(ak_sb[:], ps_a[:, 192:208])
            nc.scalar.copy(av_sb[:], ps_a[:, 208:224])
            nc.scalar.copy(bk_sb[:], ps_a[:, 224:352])
            nc.scalar.copy(bv_sb[:], ps_a[:, 352:480])
            nc.scalar.copy(bq_sb[:], ps_b[:, 0:1536])

            # ---- RoPE on B_q (DVE, bf16) ----
            t_a = tmp_pool.tile([128, 768], dt.bfloat16, tag="t_a")
            t_b = tmp_pool.tile([128, 768], dt.bfloat16, tag="t_b")
            bqv = bq_sb[:].rearrange("p (r two d) -> p r two d", r=RQ, two=2)
            bqrv = bqr[:, 0:1536].rearrange(
                "p (r two d) -> p r two d", r=RQ, two=2
            )
            cos_t = cos_sb[:, it * 768 : (it + 1) * 768].rearrange(
                "p (r d) -> p r d", r=RQ
            )
            sin_t = sin_sb[:, it * 768 : (it + 1) * 768].rearrange(
                "p (r d) -> p r d", r=RQ
            )
            tav = t_a[:].rearrange("p (r d) -> p r d", r=RQ)
            tbv = t_b[:].rearrange("p (r d) -> p r d", r=RQ)
            p_lo = bqv[:, :, 0]
            p_hi = bqv[:, :, 1]
            nc.vector.tensor_mul(tav, p_lo, cos_t)
            nc.vector.tensor_mul(tbv, p_hi, sin_t)
            nc.vector.tensor_sub(bqrv[:, :, 0], tav, tbv)
            nc.vector.tensor_mul(tav, p_hi, cos_t)
            nc.vector.tensor_mul(tbv, p_lo, sin_t)
            nc.vector.tensor_add(bqrv[:, :, 1], tav, tbv)

            # ---- RoPE on B_k (DVE, bf16) ----
            tk_a = small_pool.tile([128, 64], dt.bfloat16, tag="tk_a")
            tk_b = small_pool.tile([128, 64], dt.bfloat16, tag="tk_b")
            bkv = bk_sb[:].rearrange("p (two d) -> p two d", two=2)
            bkrv = bkr_sb[:].rearrange("p (two d) -> p two d", two=2)
            cos_k = cos_sb[:, it * 768 : it * 768 + 64]
            sin_k = sin_sb[:, it * 768 : it * 768 + 64]
            nc.vector.tensor_mul(tk_a[:], bkv[:, 0], cos_k)
            nc.vector.tensor_mul(tk_b[:], bkv[:, 1], sin_k)
            nc.vector.tensor_sub(bkrv[:, 0], tk_a[:], tk_b[:])
            nc.vector.tensor_mul(tk_a[:], bkv[:, 1], cos_k)
            nc.vector.tensor_mul(tk_b[:], bkv[:, 0], sin_k)
            nc.vector.tensor_add(bkrv[:, 1], tk_a[:], tk_b[:])

            # ---- scatter A', roped B_q into block-diagonal layout ----
            # Bounce through DRAM (partition-interleaves must keep the SBUF
            # side partition-leading); read back with ONE DMA per operand.
            scr = dram_pool.tile([128, 1728], dt.bfloat16, tag="scr_b")
            nc.scalar.dma_start(out=scr[:], in_=bqr[:])
            sa_v = scr[:, 1536:1728].rearrange(
                "(g t) (r h) -> t r g h", t=8, r=RQ
            )
            sb_v = scr[:, 0:1536].rearrange("(g t) (r d) -> t r g d", t=8, r=RQ)
            l_v = lhs[0:96, :].rearrange("(t r) (g c) -> t r g c", t=8, g=16)
            d_v = bdr[0:96, :].rearrange("(t r) (g d) -> t r g d", t=8, g=16)
            for t in range(8):
                nc.sync.dma_start(
                    out=l_v[t][:, :, t * 16 : (t + 1) * 16], in_=sa_v[t]
                )
                nc.scalar.dma_start(out=d_v[t], in_=sb_v[t])

            state[it] = (lhs, bdr, ak_sb, av_sb, bkr_sb, bv_sb)

        def consume(it):
            """q contraction + k/v broadcast + output DMAs for tile `it`."""
            t0 = it * 128
            lhs, bdr, ak_sb, av_sb, bkr_sb, bv_sb = state.pop(it)

            # ---- q: block-diagonal matmuls (PE) + PSUM evict (DVE) ----
            qsb = out_pool.tile([128, 2048], dt.bfloat16, tag="qsb")
            for gq in range(4):
                qp = psq_pool.tile([128, 512], dt.float32, tag="qp")
                for j in range(4):
                    g = gq * 4 + j
                    nc.tensor.matmul(
                        qp[:, j * 128 : (j + 1) * 128],
                        lhs[0:96, g * 128 : (g + 1) * 128],
                        bdr[0:96, g * 128 : (g + 1) * 128],
                        start=True,
                        stop=True,
                    )
                nc.vector.tensor_copy(qsb[:, gq * 512 : (gq + 1) * 512], qp[:])

            # ---- k (DVE tensor_scalar), v (ACT activation) ----
            ksb = out_pool.tile([128, 2048], dt.bfloat16, tag="ksb")
            vsb = out_pool.tile([128, 2048], dt.bfloat16, tag="vsb")
            for h in range(NH):
                nc.vector.tensor_scalar_mul(
                    ksb[:, h * 128 : (h + 1) * 128], bkr_sb[:], ak_sb[:, h : h + 1]
                )
                nc.scalar.mul(
                    vsb[:, h * 128 : (h + 1) * 128], bv_sb[:], av_sb[:, h : h + 1]
                )

            # ---- outputs (split across the two HWDGE rings) ----
            nc.sync.dma_start(
                out=q_d[t0 : t0 + 128].rearrange("(g t) h d -> (t h) g d", g=16),
                in_=qsb[:].rearrange("p (g d) -> p g d", g=16),
            )
            nc.sync.dma_start(out=k_d[t0 : t0 + 128, :], in_=ksb[:])
            nc.scalar.dma_start(out=v_d[t0 : t0 + 128, :], in_=vsb[:])

        # 1-deep software pipeline: BD matmuls of tile i issue after step-1
        # of tile i+1, so the scatter chain latency hides behind PE work.
        for it in range(NT + 1):
            if it < NT:
                produce(it)
            if it >= 1:
                consume(it - 1)


def build_program():
    import concourse.tile as tile

    nc, tensors = make_nc()
    with tile.TileContext(nc) as tc:
        build_body(nc, tc, tensors)
    nc.compile()
    return nc


def _get_program():
    if "nc" not in _CACHE:
        _CACHE["nc"] = build_program()
    return _CACHE["nc"]


def make_in_maps(x, W_A_q, W_B_q, W_A_k, W_B_k, W_A_v, W_B_v):
    """Shard + preprocess full inputs into per-core input maps."""
    x = np.asarray(x)
    B, S, Hh = x.shape
    x2 = np.ascontiguousarray(x.reshape(B * S, Hh))

    # fold the 1/RQ scale and the (h,r)->(r,h) column reorder into W_A_q
    WAq = np.asarray(W_A_q).reshape(Hh, NH, RQ).transpose(0, 2, 1).reshape(
        Hh, NH * RQ
    ) / np.float32(RQ)
    Wall = np.concatenate(
        [
            WAq,
            np.asarray(W_A_k),
            np.asarray(W_A_v),
            np.asarray(W_B_k),
            np.asarray(W_B_v),
            np.asarray(W_B_q),
        ],
        axis=1,
    )
    assert Wall.shape == (Hh, NOUT)
    Wt = np.ascontiguousarray(Wall.reshape(KT, 128, NOUT)).astype(BF16)

    inv = 1.0 / (10000.0 ** (np.arange(0, HD, 2, dtype=np.float32) / HD))
    ang = np.arange(S, dtype=np.float32)[:, None] * inv[None, :]
    cos_rep = np.ascontiguousarray(np.tile(np.cos(ang), (1, RQ))).astype(BF16)
    sin_rep = np.ascontiguousarray(np.tile(np.sin(ang), (1, RQ))).astype(BF16)

    in_maps = []
    for i in range(8):
        tok0 = i * SH
        pos = np.arange(tok0, tok0 + SH) % S
        in_maps.append(
            {
                # pre-transposed (hidden, tokens) so on-chip loads are plain
                "x": np.ascontiguousarray(x2[tok0 : tok0 + SH].T).astype(BF16),
                "w": Wt,
                "cosr": np.ascontiguousarray(cos_rep[pos]),
                "sinr": np.ascontiguousarray(sin_rep[pos]),
            }
        )
    return in_maps, (B, S)


def assemble_outputs(results, B, S):
    q = np.concatenate(
        [results[i]["q"].astype(np.float32) for i in range(8)], axis=0
    ).reshape(B, S, NH, HD)
    k = np.concatenate(
        [results[i]["k"].astype(np.float32) for i in range(8)], axis=0
    ).reshape(B, S, NH, HD)
    v = np.concatenate(
        [results[i]["v"].astype(np.float32) for i in range(8)], axis=0
    ).reshape(B, S, NH, HD)
    return q, k, v


def kernel(x, W_A_q, W_B_q, W_A_k, W_B_k, W_A_v, W_B_v):
    from concourse.bass_utils import run_bass_kernel_spmd

    nc = _get_program()
    in_maps, (B, S) = make_in_maps(x, W_A_q, W_B_q, W_A_k, W_B_k, W_A_v, W_B_v)
    res = run_bass_kernel_spmd(nc, in_maps, list(range(8))).results
    return assemble_outputs(res, B, S)
